# revision 4
# baseline (speedup 1.0000x reference)
"""BERT-BiGRU-CRF loss kernel for 8 TRN2 NeuronCores.

Strategy (per sharding hint): data-parallel over batch. Each of the 8 cores
computes the dominant GEMM - the GRU input projections for both directions,
x[16*512, 768] @ Wcat[768, 384] - on the TensorEngine via a Bass/Tile kernel
wrapped in bass_jit + bass_shard_map, so the compiled SPMD executable is
built once at module scope and reused across kernel() calls (the generic
run_bass_kernel_spmd axon path re-traces and re-lowers the NEFF wrapper on
every invocation, which dominated the baseline's wall time).

The sequential parts (GRU over T with 64-wide hidden, CRF forward with 9
labels) run on host via a jax-CPU jitted scan, exactly mirroring the
reference math; the scalar mean loss is the final reduction.
"""

import numpy as np

B, T, HID = 128, 512, 768
H = 64
G3 = 3 * H            # 192
L = 9
NCORES = 8
BS = B // NCORES      # 16 sequences per core
M = BS * T            # 8192 rows per core
N = 2 * G3            # 384: [fwd gates | bwd gates]
K = HID

_C = {}


def _build():
    """Build device + host executables once; cache in _C."""
    import jax
    try:
        jax.config.update("jax_compilation_cache_dir", "/tmp/jaxcache")
        jax.config.update("jax_persistent_cache_min_entry_size_bytes", -1)
        jax.config.update("jax_persistent_cache_min_compile_time_secs", 0)
    except Exception:
        pass
    import jax.numpy as jnp
    from jax.sharding import Mesh, PartitionSpec as P
    import concourse.mybir as mybir
    from concourse import tile
    from concourse.bass2jax import bass_jit, bass_shard_map

    f32 = mybir.dt.float32
    KT = K // 128          # 6 k-tiles
    MG = M // 512          # 16 groups of 512 rows

    @bass_jit
    def proj_kernel(nc, xT, W):
        # xT: [K, M] per-core, W: [K, N] replicated; out[M, N] = xT.T @ W
        out = nc.dram_tensor("out", [M, N], f32, kind="ExternalOutput")
        with tile.TileContext(nc) as tc:
            with (
                tc.tile_pool(name="wp", bufs=1) as wp,
                tc.tile_pool(name="xp", bufs=2) as xp,
                tc.tile_pool(name="op", bufs=4) as op,
                tc.tile_pool(name="pp", bufs=4, space="PSUM") as pp,
            ):
                w_tiles = []
                for k in range(KT):
                    wt = wp.tile([128, N], f32, tag=f"w{k}")
                    nc.sync.dma_start(wt[:], W[k * 128:(k + 1) * 128, :])
                    w_tiles.append(wt)
                for mg in range(MG):
                    x_tiles = []
                    for k in range(KT):
                        xt = xp.tile([128, 512], f32, tag=f"x{k}")
                        nc.sync.dma_start(
                            xt[:], xT[k * 128:(k + 1) * 128, mg * 512:(mg + 1) * 512]
                        )
                        x_tiles.append(xt)
                    for sub in range(4):
                        ps = pp.tile([128, N], f32, tag="ps")
                        for k in range(KT):
                            nc.tensor.matmul(
                                ps[:],
                                x_tiles[k][:, sub * 128:(sub + 1) * 128],
                                w_tiles[k][:],
                                start=(k == 0),
                                stop=(k == KT - 1),
                            )
                        ot = op.tile([128, N], f32, tag="o")
                        nc.vector.tensor_copy(ot[:], ps[:])
                        m0 = mg * 512 + sub * 128
                        nc.sync.dma_start(out[m0:m0 + 128, :], ot[:])
        return out

    devices = jax.devices()[:NCORES]
    mesh = Mesh(np.asarray(devices), ("c",))
    sharded = bass_shard_map(
        proj_kernel, mesh=mesh, in_specs=(P("c"), P()), out_specs=P("c")
    )

    # ---- host-side GRU + CRF, jitted on CPU ----
    cpu = jax.devices("cpu")[0]

    def finish(proj, mask, label, b_ih_f, b_hh_f, W_hh_f, b_ih_b, b_hh_b,
               W_hh_b, W_lin, b_lin, start_trans, end_trans, trans):
        # proj: [B,T,384] = x @ [W_ih_f.T | W_ih_b.T]
        m = mask
        mf = m.astype(jnp.float32)
        mt = mf.T[:, :, None]                                   # [T,B,1]
        xp_f = proj[:, :, :G3].transpose(1, 0, 2) + b_ih_f      # [T,B,3H]
        xp_b = proj[:, :, G3:].transpose(1, 0, 2) + b_ih_b

        def gru(xp, mtd, W_hh, b_hh):
            def step(h, inp):
                xg, mtt = inp
                hg = h @ W_hh.T + b_hh
                r = jax.nn.sigmoid(xg[:, :H] + hg[:, :H])
                z = jax.nn.sigmoid(xg[:, H:2 * H] + hg[:, H:2 * H])
                n = jnp.tanh(xg[:, 2 * H:] + r * hg[:, 2 * H:])
                h_new = (1.0 - z) * n + z * h
                h = jnp.where(mtt > 0, h_new, h)
                return h, h * mtt
            h0 = jnp.zeros((xp.shape[1], H), xp.dtype)
            _, out = jax.lax.scan(step, h0, (xp, mtd))
            return out

        out_f = gru(xp_f, mt, W_hh_f, b_hh_f)
        out_b = gru(xp_b[::-1], mt[::-1], W_hh_b, b_hh_b)[::-1]
        feat = jnp.concatenate([out_f, out_b], -1).transpose(1, 0, 2)
        em = feat @ W_lin.T + b_lin                             # [B,T,L]

        em_sc = jnp.take_along_axis(em, label[..., None], -1)[..., 0]
        tr_sc = trans[label[:, :-1], label[:, 1:]]
        score = start_trans[label[:, 0]] + em_sc[:, 0] \
            + jnp.sum(mf[:, 1:] * (tr_sc + em_sc[:, 1:]), axis=1)
        last = jnp.sum(m.astype(jnp.int32), axis=1) - 1
        last_tag = jnp.take_along_axis(label, last[:, None], 1)[:, 0]
        score = score + end_trans[last_tag]

        def pstep(alpha, inp):
            em_t, m_t = inp
            nxt = jax.nn.logsumexp(
                alpha[:, :, None] + trans[None] + em_t[:, None, :], axis=1)
            return jnp.where(m_t[:, None], nxt, alpha), None
        alpha0 = start_trans + em[:, 0]
        alpha, _ = jax.lax.scan(
            pstep, alpha0, (em[:, 1:].transpose(1, 0, 2), m[:, 1:].T))
        logZ = jax.nn.logsumexp(alpha + end_trans, axis=-1)
        return -jnp.mean(score - logZ)

    with jax.default_device(cpu):
        finish_jit = jax.jit(finish)

    _C["sharded"] = sharded
    _C["finish"] = finish_jit
    _C["cpu"] = cpu
    _C["jax"] = jax
    return _C


def kernel(length, word2vec, mask, label, W_ih_f, W_hh_f, b_ih_f, b_hh_f,
           W_ih_b, W_hh_b, b_ih_b, b_hh_b, W_lin, b_lin,
           start_trans, end_trans, trans):
    word2vec = np.asarray(word2vec, np.float32)
    mask = np.asarray(mask)
    label = np.asarray(label)
    Wcat = np.ascontiguousarray(
        np.concatenate([np.asarray(W_ih_f).T, np.asarray(W_ih_b).T], axis=1),
        dtype=np.float32)

    import time as _time
    tlog = _C.setdefault("t", {})
    proj = None
    try:
        t0 = _time.perf_counter()
        if not _C or "sharded" not in _C:
            _build()
        t1 = _time.perf_counter()
        jax = _C["jax"]
        # per-core [K, M] stacked on axis 0 -> [8*K, M]
        xT_all = np.ascontiguousarray(
            word2vec.reshape(NCORES, M, K).transpose(0, 2, 1)
        ).reshape(NCORES * K, M)
        t2 = _time.perf_counter()
        out = _C["sharded"](xT_all, Wcat)       # [8*M, N]
        out.block_until_ready()
        t3 = _time.perf_counter()
        proj = np.asarray(out).reshape(B, T, N)
        t4 = _time.perf_counter()
        tlog.update(build=t1 - t0, transpose=t2 - t1, device=t3 - t2,
                    fetch=t4 - t3, dev_ok=True)
    except Exception as e:
        tlog.update(dev_ok=False, dev_err=repr(e)[:500])
        proj = (word2vec.reshape(B * T, K) @ Wcat).reshape(B, T, N)

    try:
        t5 = _time.perf_counter()
        jax = _C["jax"]
        cpu = _C["cpu"]
        dp = lambda a: jax.device_put(np.asarray(a), cpu)
        loss = _C["finish"](
            dp(proj), dp(mask), dp(label),
            dp(b_ih_f), dp(b_hh_f), dp(W_hh_f),
            dp(b_ih_b), dp(b_hh_b), dp(W_hh_b),
            dp(W_lin), dp(b_lin),
            dp(start_trans), dp(end_trans), dp(trans))
        loss = np.float32(loss)
        tlog.update(finish=_time.perf_counter() - t5, fin_ok=True)
        return loss
    except Exception as e:
        tlog.update(fin_ok=False, fin_err=repr(e)[:500])
        return _finish_np(
            proj, mask, label,
            np.asarray(b_ih_f), np.asarray(b_hh_f), np.asarray(W_hh_f),
            np.asarray(b_ih_b), np.asarray(b_hh_b), np.asarray(W_hh_b),
            np.asarray(W_lin), np.asarray(b_lin),
            np.asarray(start_trans), np.asarray(end_trans), np.asarray(trans))


# ---------- pure-numpy fallback (mirrors reference exactly) ----------

def _sigmoid(x):
    return 1.0 / (1.0 + np.exp(-x))


def _gru_dir_np(xp, m, W_hh, b_hh):
    Bn = xp.shape[1]
    h = np.zeros((Bn, H), np.float32)
    out = np.empty((T, Bn, H), np.float32)
    WhhT = W_hh.T.astype(np.float32)
    for t in range(T):
        hg = h @ WhhT + b_hh
        xg = xp[t]
        r = _sigmoid(xg[:, :H] + hg[:, :H])
        z = _sigmoid(xg[:, H:2 * H] + hg[:, H:2 * H])
        n = np.tanh(xg[:, 2 * H:] + r * hg[:, 2 * H:])
        h_new = (1.0 - z) * n + z * h
        mt = m[t]
        h = np.where(mt > 0, h_new, h)
        out[t] = h * mt
    return out


def _logsumexp_np(x, axis):
    mx = np.max(x, axis=axis, keepdims=True)
    return (mx + np.log(np.sum(np.exp(x - mx), axis=axis, keepdims=True))).squeeze(axis)


def _finish_np(proj, mask, label, b_ih_f, b_hh_f, W_hh_f,
               b_ih_b, b_hh_b, W_hh_b, W_lin, b_lin,
               start_trans, end_trans, trans):
    mf = mask.astype(np.float32)
    mt = mf.T[:, :, None]
    xp_f = proj[:, :, :G3].transpose(1, 0, 2) + b_ih_f
    xp_b = proj[:, :, G3:].transpose(1, 0, 2) + b_ih_b
    out_f = _gru_dir_np(xp_f, mt, W_hh_f, b_hh_f)
    out_b = _gru_dir_np(xp_b[::-1], mt[::-1], W_hh_b, b_hh_b)[::-1]
    feat = np.concatenate([out_f, out_b], -1).transpose(1, 0, 2)
    em = feat @ W_lin.T + b_lin

    em_sc = np.take_along_axis(em, label[..., None], -1)[..., 0]
    tr_sc = trans[label[:, :-1], label[:, 1:]]
    score = start_trans[label[:, 0]] + em_sc[:, 0] \
        + np.sum(mf[:, 1:] * (tr_sc + em_sc[:, 1:]), axis=1)
    last = mask.astype(np.int64).sum(1) - 1
    last_tag = label[np.arange(label.shape[0]), last]
    score = score + end_trans[last_tag]

    alpha = start_trans + em[:, 0]
    for t in range(1, T):
        nxt = _logsumexp_np(
            alpha[:, :, None] + trans[None] + em[:, t][:, None, :], axis=1)
        alpha = np.where(mask[:, t][:, None], nxt, alpha)
    logZ = _logsumexp_np(alpha + end_trans, axis=-1)
    return np.float32(-(score - logZ).mean())


# revision 19
# speedup vs baseline: 2.0174x; 2.0174x over previous
"""BERT-BiGRU-CRF loss kernel for 8 TRN2 NeuronCores.

Data-parallel over batch (16 sequences per core). The axon tunnel moves
~50-90 MB/s each way, so the design minimizes transfer bytes:

  host:   permute x to [t*16+b, 768] rows, cast bf16       (100 MB up)
  device: PE-transpose x tiles -> GEMM input projections (xpT, transposed
          gate layout [gate-dim on partitions, (t,b) on free]) -> fused
          fwd+bwd GRU recurrence (both directions stacked on partitions,
          512 steps) -> emissions GEMM -> em [8192, 9] f32   (2.4 MB down)
  host:   CRF forward + gold score + mean loss (jax CPU jit)

The whole device program is one Bass/Tile kernel wrapped in bass_jit +
bass_shard_map; the jitted SPMD executable is built once at module scope
and cached across kernel() calls (re-tracing per call dominated the
original baseline). jax persistent compilation cache avoids recompiling
the NEFF across processes.
"""

import numpy as np

B, T, HID = 128, 512, 768
H = 64                # per-direction GRU hidden
G3 = 3 * H            # 192 gates per direction
L = 9
NCORES = 8
BS = B // NCORES      # 16 sequences per core
M = BS * T            # 8192 rows per core (m' = t*16 + b)
N = 2 * G3            # 384 gate columns
K = HID
CHUNKS = 64           # em chunks of 128 rows

_C = {}


X_DTYPE = "bf16"          # "bf16" or "fp8" for the x upload / GEMM input path


def _build_device_kernel(xdt=None):
    import concourse.mybir as mybir
    from concourse import tile
    from concourse.bass2jax import bass_jit

    f32 = mybir.dt.float32
    bf16 = mybir.dt.bfloat16
    xd = mybir.dt.float8e4 if (xdt or X_DTYPE) == "fp8" else bf16
    from concourse.alu_op_type import AluOpType as ALU
    import bass_rust
    ACT_F = bass_rust.ActivationFunctionType

    @bass_jit
    def bigru_kernel(nc, x, wk, wr, params, mrow, ident):
        # x:      [M, K] bf16, rows m' = t*16 + b
        # wk:     [K, N] bf16, cols [rf rb | zf zb | nf nb] per 128-chunk
        # wr:     [128, 384] f32 block-diag recurrence weights (lhsT layout)
        # params: [128, 13] f32: col0-2 gemm bias per chunk, col3 b_hh_n,
        #         cols 4:13 W_lin.T
        # mrow:   [1, M] f32 mask at col t*16+b
        # ident:  [128, 128] bf16 identity (PE transpose operand)
        em_out = nc.dram_tensor("em_out", [128, CHUNKS * L], f32,
                                kind="ExternalOutput")
        KT = K // 128      # 6
        MG = M // 512      # 16
        with tile.TileContext(nc) as tc:
            with (
                tc.tile_pool(name="const", bufs=1) as cp,
                tc.tile_pool(name="xin", bufs=8) as xin,
                tc.tile_pool(name="xtp", bufs=2) as xtp,
                tc.tile_pool(name="ps_t", bufs=2, space="PSUM") as ps_t,
                tc.tile_pool(name="ps_g", bufs=2, space="PSUM") as ps_g,
                tc.tile_pool(name="ps_r", bufs=4, space="PSUM") as ps_r,
                tc.tile_pool(name="gtmp", bufs=3) as gtmp,
            ):
                # ---- persistent tiles ----
                idn = cp.tile([128, 128], xd, tag="idn")
                nc.sync.dma_start(idn[:], ident[:, :])
                w_sb = []
                for k in range(KT):
                    wt = cp.tile([128, N], xd, tag=f"wk{k}")
                    nc.sync.dma_start(wt[:], wk[k * 128:(k + 1) * 128, :])
                    w_sb.append(wt)
                wr_sb = cp.tile([128, 384], f32, tag="wr")
                nc.sync.dma_start(wr_sb[:], wr[:, :])
                par = cp.tile([128, 13], f32, tag="par")
                nc.sync.dma_start(par[:], params[:, :])
                mrow_sb = cp.tile([1, M], bf16, tag="mrow")
                nc.sync.dma_start(mrow_sb[:], mrow[:, :])
                ones = cp.tile([1, 128], bf16, tag="ones")
                nc.vector.memset(ones[:], 1.0)

                maskrep = cp.tile([128, M], bf16, tag="mrep")
                xpT = [cp.tile([128, M], f32, tag=f"xpT{c}", name=f"xpT{c}")
                       for c in range(3)]
                featT = cp.tile([128, M], f32, tag="featT")
                h = cp.tile([128, BS], f32, tag="h")
                nc.vector.memset(h[:], 0.0)
                em_sb = cp.tile([128, CHUNKS * L], f32, tag="em")

                # ---- A: replicate mask across partitions via K=1 matmul ----
                for j in range(MG):
                    pm = ps_g.tile([128, 512], f32, tag="pg")
                    nc.tensor.matmul(
                        pm[:], ones[:], mrow_sb[:, j * 512:(j + 1) * 512],
                        start=True, stop=True)
                    nc.vector.tensor_copy(maskrep[:, j * 512:(j + 1) * 512], pm[:])

                # ---- B: input projection GEMM (with PE transpose of x) ----
                for mg in range(MG):
                    xrows = []
                    for s in range(4):
                        xr = xin.tile([128, K], xd, tag="xr")
                        r0 = mg * 512 + s * 128
                        nc.sync.dma_start(xr[:], x[r0:r0 + 128, :])
                        xrows.append(xr)
                    xT = []
                    for k in range(KT):
                        xk = xtp.tile([128, 512], xd, tag=f"xT{k}")
                        for s in range(4):
                            pt = ps_t.tile([128, 128], xd, tag="pt")
                            nc.tensor.transpose(
                                pt[:], xrows[s][:, k * 128:(k + 1) * 128], idn[:])
                            nc.vector.tensor_copy(
                                xk[:, s * 128:(s + 1) * 128], pt[:])
                        xT.append(xk)
                    for c in range(3):
                        pg = ps_g.tile([128, 512], f32, tag="pg")
                        for k in range(KT):
                            nc.tensor.matmul(
                                pg[:],
                                w_sb[k][:, c * 128:(c + 1) * 128],
                                xT[k][:],
                                start=(k == 0), stop=(k == KT - 1))
                        nc.scalar.activation(
                            xpT[c][:, mg * 512:(mg + 1) * 512], pg[:],
                            ACT_F.Identity, bias=par[:, c:c + 1], scale=1.0)

                # ---- C: fused fwd+bwd GRU, 512 steps ----
                # partitions 0:64 = forward dir, 64:128 = backward dir
                bhn = par[:, 3:4]
                for s in range(T):
                    tf, tb = s, T - 1 - s
                    cf = slice(tf * BS, (tf + 1) * BS)
                    cb = slice(tb * BS, (tb + 1) * BS)
                    psR = ps_r.tile([128, BS], f32, tag="pr")
                    psZ = ps_r.tile([128, BS], f32, tag="pr")
                    psN = ps_r.tile([128, BS], f32, tag="pr")
                    nc.tensor.matmul(psR[:], wr_sb[:, 0:128], h[:],
                                     start=True, stop=True)
                    nc.tensor.matmul(psZ[:], wr_sb[:, 128:256], h[:],
                                     start=True, stop=True)
                    nc.tensor.matmul(psN[:], wr_sb[:, 256:384], h[:],
                                     start=True, stop=True)
                    tr_ = gtmp.tile([128, BS], f32, tag="tr")
                    nc.vector.tensor_tensor(
                        tr_[0:64, :], psR[0:64, :], xpT[0][0:64, cf], ALU.add)
                    nc.vector.tensor_tensor(
                        tr_[64:128, :], psR[64:128, :], xpT[0][64:128, cb], ALU.add)
                    r = gtmp.tile([128, BS], f32, tag="r")
                    nc.scalar.activation(r[:], tr_[:], ACT_F.Sigmoid)
                    tz = gtmp.tile([128, BS], f32, tag="tz")
                    nc.vector.tensor_tensor(
                        tz[0:64, :], psZ[0:64, :], xpT[1][0:64, cf], ALU.add)
                    nc.vector.tensor_tensor(
                        tz[64:128, :], psZ[64:128, :], xpT[1][64:128, cb], ALU.add)
                    z = gtmp.tile([128, BS], f32, tag="z")
                    nc.scalar.activation(z[:], tz[:], ACT_F.Sigmoid)
                    # w = m - z*m  (per-direction mask columns)
                    zm = gtmp.tile([128, BS], f32, tag="zm")
                    nc.vector.tensor_tensor(
                        zm[0:64, :], z[0:64, :], maskrep[0:64, cf], ALU.mult)
                    nc.vector.tensor_tensor(
                        zm[64:128, :], z[64:128, :], maskrep[64:128, cb], ALU.mult)
                    w = gtmp.tile([128, BS], f32, tag="w")
                    nc.vector.tensor_tensor(
                        w[0:64, :], maskrep[0:64, cf], zm[0:64, :], ALU.subtract)
                    nc.vector.tensor_tensor(
                        w[64:128, :], maskrep[64:128, cb], zm[64:128, :],
                        ALU.subtract)
                    # n = tanh(xp_n + r*(psN + b_hh_n))
                    t1 = gtmp.tile([128, BS], f32, tag="t1")
                    nc.vector.scalar_tensor_tensor(
                        t1[:], psN[:], bhn, r[:], ALU.add, ALU.mult)
                    t2 = gtmp.tile([128, BS], f32, tag="t2")
                    nc.vector.tensor_tensor(
                        t2[0:64, :], t1[0:64, :], xpT[2][0:64, cf], ALU.add)
                    nc.vector.tensor_tensor(
                        t2[64:128, :], t1[64:128, :], xpT[2][64:128, cb], ALU.add)
                    n = gtmp.tile([128, BS], f32, tag="n")
                    nc.scalar.activation(n[:], t2[:], ACT_F.Tanh)
                    # h += w * (n - h);  out_t = h * m
                    a = gtmp.tile([128, BS], f32, tag="a")
                    nc.vector.tensor_tensor(a[:], n[:], h[:], ALU.subtract)
                    am = gtmp.tile([128, BS], f32, tag="am")
                    nc.vector.tensor_tensor(am[:], a[:], w[:], ALU.mult)
                    nc.vector.tensor_tensor(h[:], h[:], am[:], ALU.add)
                    nc.vector.tensor_tensor(
                        featT[0:64, cf], h[0:64, :], maskrep[0:64, cf], ALU.mult)
                    nc.vector.tensor_tensor(
                        featT[64:128, cb], h[64:128, :], maskrep[64:128, cb],
                        ALU.mult)

                # ---- D: emissions GEMM em[m', L] ----
                for c in range(CHUNKS):
                    pe = ps_r.tile([128, L], f32, tag="pr")
                    nc.tensor.matmul(
                        pe[:], featT[:, c * 128:(c + 1) * 128], par[:, 4:4 + L],
                        start=True, stop=True)
                    nc.vector.tensor_copy(em_sb[:, c * L:(c + 1) * L], pe[:])
                nc.sync.dma_start(em_out[:, :], em_sb[:])
        return em_out

    return bigru_kernel


def _build():
    """Build device + host executables once; cache in _C."""
    import jax
    try:
        jax.config.update("jax_compilation_cache_dir", "/tmp/jaxcache")
        jax.config.update("jax_persistent_cache_min_entry_size_bytes", -1)
        jax.config.update("jax_persistent_cache_min_compile_time_secs", 0)
    except Exception:
        pass
    import jax.numpy as jnp
    from jax.sharding import Mesh, PartitionSpec as P
    from concourse.bass2jax import bass_shard_map

    bigru_kernel = _build_device_kernel()
    devices = jax.devices()[:NCORES]
    mesh = Mesh(np.asarray(devices), ("c",))
    sharded = bass_shard_map(
        bigru_kernel, mesh=mesh,
        in_specs=(P("c"), P(), P(), P(), P("c"), P()),
        out_specs=P("c"))

    cpu = jax.devices("cpu")[0]

    # host prep of x: [B,T,K] f32 -> [8*M, K] rows m'=t*16+b, cast
    xdt_np = _xdt_np()

    def prep_x(w):
        wp = jnp.transpose(w.reshape(NCORES, BS, T, K), (0, 2, 1, 3))
        return wp.astype(xdt_np).reshape(NCORES * M, K)

    # host CRF + score from emissions
    def crf(em, mask, label, b_lin, start_trans, end_trans, trans):
        em = em + b_lin
        m = mask
        mf = m.astype(jnp.float32)
        em_sc = jnp.take_along_axis(em, label[..., None], -1)[..., 0]
        tr_sc = trans[label[:, :-1], label[:, 1:]]
        score = start_trans[label[:, 0]] + em_sc[:, 0] \
            + jnp.sum(mf[:, 1:] * (tr_sc + em_sc[:, 1:]), axis=1)
        last = jnp.sum(m.astype(jnp.int32), axis=1) - 1
        last_tag = jnp.take_along_axis(label, last[:, None], 1)[:, 0]
        score = score + end_trans[last_tag]

        def pstep(alpha, inp):
            em_t, m_t = inp
            nxt = jax.nn.logsumexp(
                alpha[:, :, None] + trans[None] + em_t[:, None, :], axis=1)
            return jnp.where(m_t[:, None], nxt, alpha), None
        alpha0 = start_trans + em[:, 0]
        alpha, _ = jax.lax.scan(
            pstep, alpha0, (em[:, 1:].transpose(1, 0, 2), m[:, 1:].T))
        logZ = jax.nn.logsumexp(alpha + end_trans, axis=-1)
        return -jnp.mean(score - logZ)

    with jax.default_device(cpu):
        prep_jit = jax.jit(prep_x)
        crf_jit = jax.jit(crf)

    _C.update(sharded=sharded, prep=prep_jit, crf=crf_jit, cpu=cpu, jax=jax)
    return _C


def _xdt_np():
    import ml_dtypes
    return ml_dtypes.float8_e4m3 if X_DTYPE == "fp8" else ml_dtypes.bfloat16


def _host_params(W_ih_f, W_ih_b, W_hh_f, W_hh_b, b_ih_f, b_ih_b,
                 b_hh_f, b_hh_b, W_lin):
    """Build wk [K,N], wr [128,384] f32, params [128,13] f32."""
    xdt = _xdt_np()
    wk = np.empty((K, N), np.float32)
    for c in range(3):          # r, z, n
        wk[:, c * 128:c * 128 + 64] = W_ih_f[c * 64:(c + 1) * 64, :].T
        wk[:, c * 128 + 64:(c + 1) * 128] = W_ih_b[c * 64:(c + 1) * 64, :].T
    wk = wk.astype(xdt)

    wr = np.zeros((128, 384), np.float32)
    for c in range(3):
        wr[0:64, c * 128:c * 128 + 64] = W_hh_f[c * 64:(c + 1) * 64, :].T
        wr[64:128, c * 128 + 64:(c + 1) * 128] = W_hh_b[c * 64:(c + 1) * 64, :].T

    params = np.zeros((128, 13), np.float32)
    for c in range(3):
        bf = b_ih_f[c * 64:(c + 1) * 64].copy()
        bb = b_ih_b[c * 64:(c + 1) * 64].copy()
        if c < 2:               # fold b_hh into r,z; n keeps b_ih only
            bf += b_hh_f[c * 64:(c + 1) * 64]
            bb += b_hh_b[c * 64:(c + 1) * 64]
        params[0:64, c] = bf
        params[64:128, c] = bb
    params[0:64, 3] = b_hh_f[128:192]
    params[64:128, 3] = b_hh_b[128:192]
    params[:, 4:4 + L] = W_lin.T
    ident = np.eye(128, dtype=xdt)
    return wk, wr, params, ident


def kernel(length, word2vec, mask, label, W_ih_f, W_hh_f, b_ih_f, b_hh_f,
           W_ih_b, W_hh_b, b_ih_b, b_hh_b, W_lin, b_lin,
           start_trans, end_trans, trans):
    import time as _time
    word2vec = np.asarray(word2vec, np.float32)
    mask = np.asarray(mask)
    label = np.asarray(label)
    args = [np.asarray(a, np.float32) for a in
            (W_ih_f, W_hh_f, b_ih_f, b_hh_f, W_ih_b, W_hh_b, b_ih_b, b_hh_b,
             W_lin, b_lin, start_trans, end_trans, trans)]
    (W_ih_f, W_hh_f, b_ih_f, b_hh_f, W_ih_b, W_hh_b, b_ih_b, b_hh_b,
     W_lin, b_lin, start_trans, end_trans, trans) = args

    tlog = _C.setdefault("t", {})
    try:
        t0 = _time.perf_counter()
        if "sharded" not in _C:
            _build()
        jax = _C["jax"]
        t1 = _time.perf_counter()
        import ml_dtypes
        wkb, wr, params, ident = _host_params(
            W_ih_f, W_ih_b, W_hh_f, W_hh_b, b_ih_f, b_ih_b, b_hh_f, b_hh_b,
            W_lin)
        mrow = np.ascontiguousarray(
            mask.reshape(NCORES, BS, T).transpose(0, 2, 1)
        ).reshape(NCORES, 1, M).astype(ml_dtypes.bfloat16)
        xb = _C["prep"](jax.device_put(word2vec, _C["cpu"]))  # [8*M, K] on cpu
        t2 = _time.perf_counter()
        em_dev = _C["sharded"](xb, wkb, wr, params,
                               mrow.reshape(NCORES * 1, M), ident)
        em_dev.block_until_ready()
        t3 = _time.perf_counter()
        em_np = np.asarray(em_dev)          # [8*128, 576]
        t4 = _time.perf_counter()
        em = em_np.reshape(NCORES, 128, CHUNKS, L).transpose(0, 2, 1, 3)
        em = em.reshape(NCORES, T, BS, L).transpose(0, 2, 1, 3)
        em = np.ascontiguousarray(em.reshape(B, T, L))
        t5 = _time.perf_counter()
        dp = lambda a: jax.device_put(np.asarray(a), _C["cpu"])
        loss = np.float32(_C["crf"](
            dp(em), dp(mask), dp(label), dp(b_lin), dp(start_trans),
            dp(end_trans), dp(trans)))
        t6 = _time.perf_counter()
        tlog.update(build=t1 - t0, prep=t2 - t1, device=t3 - t2,
                    fetch=t4 - t3, reorder=t5 - t4, crf=t6 - t5, dev_ok=True)
        return loss
    except Exception as e:
        tlog.update(dev_ok=False, dev_err=repr(e)[:800])
        return _full_numpy(
            word2vec, mask, label, W_ih_f, W_hh_f, b_ih_f, b_hh_f,
            W_ih_b, W_hh_b, b_ih_b, b_hh_b, W_lin, b_lin,
            start_trans, end_trans, trans)


# ---------- pure-numpy fallback (mirrors reference exactly) ----------

def _sigmoid(x):
    return 1.0 / (1.0 + np.exp(-x))


def _gru_dir_np(xp, m, W_hh, b_hh):
    Bn = xp.shape[1]
    h = np.zeros((Bn, H), np.float32)
    out = np.empty((T, Bn, H), np.float32)
    WhhT = W_hh.T.astype(np.float32)
    for t in range(T):
        hg = h @ WhhT + b_hh
        xg = xp[t]
        r = _sigmoid(xg[:, :H] + hg[:, :H])
        z = _sigmoid(xg[:, H:2 * H] + hg[:, H:2 * H])
        n = np.tanh(xg[:, 2 * H:] + r * hg[:, 2 * H:])
        h_new = (1.0 - z) * n + z * h
        mt = m[t]
        h = np.where(mt > 0, h_new, h)
        out[t] = h * mt
    return out


def _logsumexp_np(x, axis):
    mx = np.max(x, axis=axis, keepdims=True)
    return (mx + np.log(np.sum(np.exp(x - mx), axis=axis,
                               keepdims=True))).squeeze(axis)


def _full_numpy(word2vec, mask, label, W_ih_f, W_hh_f, b_ih_f, b_hh_f,
                W_ih_b, W_hh_b, b_ih_b, b_hh_b, W_lin, b_lin,
                start_trans, end_trans, trans):
    Wcat = np.concatenate([W_ih_f.T, W_ih_b.T], axis=1)
    proj = (word2vec.reshape(B * T, K) @ Wcat).reshape(B, T, 2 * G3)
    mf = mask.astype(np.float32)
    mt = mf.T[:, :, None]
    xp_f = proj[:, :, :G3].transpose(1, 0, 2) + b_ih_f
    xp_b = proj[:, :, G3:].transpose(1, 0, 2) + b_ih_b
    out_f = _gru_dir_np(xp_f, mt, W_hh_f, b_hh_f)
    out_b = _gru_dir_np(xp_b[::-1], mt[::-1], W_hh_b, b_hh_b)[::-1]
    feat = np.concatenate([out_f, out_b], -1).transpose(1, 0, 2)
    em = feat @ W_lin.T + b_lin

    em_sc = np.take_along_axis(em, label[..., None], -1)[..., 0]
    tr_sc = trans[label[:, :-1], label[:, 1:]]
    score = start_trans[label[:, 0]] + em_sc[:, 0] \
        + np.sum(mf[:, 1:] * (tr_sc + em_sc[:, 1:]), axis=1)
    last = mask.astype(np.int64).sum(1) - 1
    last_tag = label[np.arange(label.shape[0]), last]
    score = score + end_trans[last_tag]

    alpha = start_trans + em[:, 0]
    for t in range(1, T):
        nxt = _logsumexp_np(
            alpha[:, :, None] + trans[None] + em[:, t][:, None, :], axis=1)
        alpha = np.where(mask[:, t][:, None], nxt, alpha)
    logZ = _logsumexp_np(alpha + end_trans, axis=-1)
    return np.float32(-(score - logZ).mean())


# revision 20
# speedup vs baseline: 9.3568x; 4.6382x over previous
"""BERT-BiGRU-CRF loss kernel for 8 TRN2 NeuronCores.

Data-parallel over batch (16 sequences per core). The axon tunnel moves
~50-90 MB/s each way, so the design minimizes transfer bytes:

  host:   permute x to [t*16+b, 768] rows, cast bf16       (100 MB up)
  device: PE-transpose x tiles -> GEMM input projections (xpT, transposed
          gate layout [gate-dim on partitions, (t,b) on free]) -> fused
          fwd+bwd GRU recurrence (both directions stacked on partitions,
          512 steps) -> emissions GEMM -> em [8192, 9] f32   (2.4 MB down)
  host:   CRF forward + gold score + mean loss (jax CPU jit)

The whole device program is one Bass/Tile kernel wrapped in bass_jit +
bass_shard_map; the jitted SPMD executable is built once at module scope
and cached across kernel() calls (re-tracing per call dominated the
original baseline). jax persistent compilation cache avoids recompiling
the NEFF across processes.
"""

import numpy as np

B, T, HID = 128, 512, 768
H = 64                # per-direction GRU hidden
G3 = 3 * H            # 192 gates per direction
L = 9
NCORES = 8
BS = B // NCORES      # 16 sequences per core
M = BS * T            # 8192 rows per core (m' = t*16 + b)
N = 2 * G3            # 384 gate columns
K = HID
CHUNKS = 64           # em chunks of 128 rows

_C = {}


X_DTYPE = "fp8"           # "bf16" or "fp8" for the x upload / GEMM input path


def _build_device_kernel(xdt=None):
    import concourse.mybir as mybir
    from concourse import tile
    from concourse.bass2jax import bass_jit

    f32 = mybir.dt.float32
    bf16 = mybir.dt.bfloat16
    xd = mybir.dt.float8e4 if (xdt or X_DTYPE) == "fp8" else bf16
    from concourse.alu_op_type import AluOpType as ALU
    import bass_rust
    ACT_F = bass_rust.ActivationFunctionType

    @bass_jit
    def bigru_kernel(nc, x, wk, wr, params, mrow, ident):
        # x:      [M, K] bf16, rows m' = t*16 + b
        # wk:     [K, N] bf16, cols [rf rb | zf zb | nf nb] per 128-chunk
        # wr:     [128, 384] f32 block-diag recurrence weights (lhsT layout)
        # params: [128, 13] f32: col0-2 gemm bias per chunk, col3 b_hh_n,
        #         cols 4:13 W_lin.T
        # mrow:   [1, M] f32 mask at col t*16+b
        # ident:  [128, 128] bf16 identity (PE transpose operand)
        em_out = nc.dram_tensor("em_out", [128, CHUNKS * L], f32,
                                kind="ExternalOutput")
        KT = K // 128      # 6
        MG = M // 512      # 16
        with tile.TileContext(nc) as tc:
            with (
                tc.tile_pool(name="const", bufs=1) as cp,
                tc.tile_pool(name="xin", bufs=8) as xin,
                tc.tile_pool(name="xtp", bufs=2) as xtp,
                tc.tile_pool(name="ps_t", bufs=2, space="PSUM") as ps_t,
                tc.tile_pool(name="ps_g", bufs=2, space="PSUM") as ps_g,
                tc.tile_pool(name="ps_r", bufs=4, space="PSUM") as ps_r,
                tc.tile_pool(name="gtmp", bufs=3) as gtmp,
            ):
                # ---- persistent tiles ----
                idn = cp.tile([128, 128], xd, tag="idn")
                nc.sync.dma_start(idn[:], ident[:, :])
                w_sb = []
                for k in range(KT):
                    wt = cp.tile([128, N], xd, tag=f"wk{k}")
                    nc.sync.dma_start(wt[:], wk[k * 128:(k + 1) * 128, :])
                    w_sb.append(wt)
                wr_sb = cp.tile([128, 384], f32, tag="wr")
                nc.sync.dma_start(wr_sb[:], wr[:, :])
                par = cp.tile([128, 13], f32, tag="par")
                nc.sync.dma_start(par[:], params[:, :])
                mrow_sb = cp.tile([1, M], bf16, tag="mrow")
                nc.sync.dma_start(mrow_sb[:], mrow[:, :])
                ones = cp.tile([1, 128], bf16, tag="ones")
                nc.vector.memset(ones[:], 1.0)

                maskrep = cp.tile([128, M], bf16, tag="mrep")
                xpT = [cp.tile([128, M], f32, tag=f"xpT{c}", name=f"xpT{c}")
                       for c in range(3)]
                featT = cp.tile([128, M], f32, tag="featT")
                h = cp.tile([128, BS], f32, tag="h")
                nc.vector.memset(h[:], 0.0)
                em_sb = cp.tile([128, CHUNKS * L], f32, tag="em")

                # ---- A: replicate mask across partitions via K=1 matmul ----
                for j in range(MG):
                    pm = ps_g.tile([128, 512], f32, tag="pg")
                    nc.tensor.matmul(
                        pm[:], ones[:], mrow_sb[:, j * 512:(j + 1) * 512],
                        start=True, stop=True)
                    nc.vector.tensor_copy(maskrep[:, j * 512:(j + 1) * 512], pm[:])

                # ---- B: input projection GEMM (with PE transpose of x) ----
                for mg in range(MG):
                    xrows = []
                    for s in range(4):
                        xr = xin.tile([128, K], xd, tag="xr")
                        r0 = mg * 512 + s * 128
                        nc.sync.dma_start(xr[:], x[r0:r0 + 128, :])
                        xrows.append(xr)
                    xT = []
                    for k in range(KT):
                        xk = xtp.tile([128, 512], xd, tag=f"xT{k}")
                        for s in range(4):
                            pt = ps_t.tile([128, 128], xd, tag="pt")
                            nc.tensor.transpose(
                                pt[:], xrows[s][:, k * 128:(k + 1) * 128], idn[:])
                            nc.vector.tensor_copy(
                                xk[:, s * 128:(s + 1) * 128], pt[:])
                        xT.append(xk)
                    for c in range(3):
                        pg = ps_g.tile([128, 512], f32, tag="pg")
                        for k in range(KT):
                            nc.tensor.matmul(
                                pg[:],
                                w_sb[k][:, c * 128:(c + 1) * 128],
                                xT[k][:],
                                start=(k == 0), stop=(k == KT - 1))
                        nc.scalar.activation(
                            xpT[c][:, mg * 512:(mg + 1) * 512], pg[:],
                            ACT_F.Identity, bias=par[:, c:c + 1], scale=1.0)

                # ---- C: fused fwd+bwd GRU, 512 steps ----
                # partitions 0:64 = forward dir, 64:128 = backward dir
                bhn = par[:, 3:4]
                for s in range(T):
                    tf, tb = s, T - 1 - s
                    cf = slice(tf * BS, (tf + 1) * BS)
                    cb = slice(tb * BS, (tb + 1) * BS)
                    psR = ps_r.tile([128, BS], f32, tag="pr")
                    psZ = ps_r.tile([128, BS], f32, tag="pr")
                    psN = ps_r.tile([128, BS], f32, tag="pr")
                    nc.tensor.matmul(psR[:], wr_sb[:, 0:128], h[:],
                                     start=True, stop=True)
                    nc.tensor.matmul(psZ[:], wr_sb[:, 128:256], h[:],
                                     start=True, stop=True)
                    nc.tensor.matmul(psN[:], wr_sb[:, 256:384], h[:],
                                     start=True, stop=True)
                    tr_ = gtmp.tile([128, BS], f32, tag="tr")
                    nc.vector.tensor_tensor(
                        tr_[0:64, :], psR[0:64, :], xpT[0][0:64, cf], ALU.add)
                    nc.vector.tensor_tensor(
                        tr_[64:128, :], psR[64:128, :], xpT[0][64:128, cb], ALU.add)
                    r = gtmp.tile([128, BS], f32, tag="r")
                    nc.scalar.activation(r[:], tr_[:], ACT_F.Sigmoid)
                    tz = gtmp.tile([128, BS], f32, tag="tz")
                    nc.vector.tensor_tensor(
                        tz[0:64, :], psZ[0:64, :], xpT[1][0:64, cf], ALU.add)
                    nc.vector.tensor_tensor(
                        tz[64:128, :], psZ[64:128, :], xpT[1][64:128, cb], ALU.add)
                    z = gtmp.tile([128, BS], f32, tag="z")
                    nc.scalar.activation(z[:], tz[:], ACT_F.Sigmoid)
                    # w = m - z*m  (per-direction mask columns)
                    zm = gtmp.tile([128, BS], f32, tag="zm")
                    nc.vector.tensor_tensor(
                        zm[0:64, :], z[0:64, :], maskrep[0:64, cf], ALU.mult)
                    nc.vector.tensor_tensor(
                        zm[64:128, :], z[64:128, :], maskrep[64:128, cb], ALU.mult)
                    w = gtmp.tile([128, BS], f32, tag="w")
                    nc.vector.tensor_tensor(
                        w[0:64, :], maskrep[0:64, cf], zm[0:64, :], ALU.subtract)
                    nc.vector.tensor_tensor(
                        w[64:128, :], maskrep[64:128, cb], zm[64:128, :],
                        ALU.subtract)
                    # n = tanh(xp_n + r*(psN + b_hh_n))
                    t1 = gtmp.tile([128, BS], f32, tag="t1")
                    nc.vector.scalar_tensor_tensor(
                        t1[:], psN[:], bhn, r[:], ALU.add, ALU.mult)
                    t2 = gtmp.tile([128, BS], f32, tag="t2")
                    nc.vector.tensor_tensor(
                        t2[0:64, :], t1[0:64, :], xpT[2][0:64, cf], ALU.add)
                    nc.vector.tensor_tensor(
                        t2[64:128, :], t1[64:128, :], xpT[2][64:128, cb], ALU.add)
                    n = gtmp.tile([128, BS], f32, tag="n")
                    nc.scalar.activation(n[:], t2[:], ACT_F.Tanh)
                    # h += w * (n - h);  out_t = h * m
                    a = gtmp.tile([128, BS], f32, tag="a")
                    nc.vector.tensor_tensor(a[:], n[:], h[:], ALU.subtract)
                    am = gtmp.tile([128, BS], f32, tag="am")
                    nc.vector.tensor_tensor(am[:], a[:], w[:], ALU.mult)
                    nc.vector.tensor_tensor(h[:], h[:], am[:], ALU.add)
                    nc.vector.tensor_tensor(
                        featT[0:64, cf], h[0:64, :], maskrep[0:64, cf], ALU.mult)
                    nc.vector.tensor_tensor(
                        featT[64:128, cb], h[64:128, :], maskrep[64:128, cb],
                        ALU.mult)

                # ---- D: emissions GEMM em[m', L] ----
                for c in range(CHUNKS):
                    pe = ps_r.tile([128, L], f32, tag="pr")
                    nc.tensor.matmul(
                        pe[:], featT[:, c * 128:(c + 1) * 128], par[:, 4:4 + L],
                        start=True, stop=True)
                    nc.vector.tensor_copy(em_sb[:, c * L:(c + 1) * L], pe[:])
                nc.sync.dma_start(em_out[:, :], em_sb[:])
        return em_out

    return bigru_kernel


def _build():
    """Build device + host executables once; cache in _C."""
    import jax
    try:
        jax.config.update("jax_compilation_cache_dir", "/tmp/jaxcache")
        jax.config.update("jax_persistent_cache_min_entry_size_bytes", -1)
        jax.config.update("jax_persistent_cache_min_compile_time_secs", 0)
    except Exception:
        pass
    import jax.numpy as jnp
    from jax.sharding import Mesh, PartitionSpec as P
    from concourse.bass2jax import bass_shard_map

    bigru_kernel = _build_device_kernel()
    devices = jax.devices()[:NCORES]
    mesh = Mesh(np.asarray(devices), ("c",))
    sharded = bass_shard_map(
        bigru_kernel, mesh=mesh,
        in_specs=(P("c"), P(), P(), P(), P("c"), P()),
        out_specs=P("c"))

    cpu = jax.devices("cpu")[0]

    # host prep of x: [B,T,K] f32 -> [8*M, K] rows m'=t*16+b, cast
    xdt_np = _xdt_np()

    def prep_x(w):
        wp = jnp.transpose(w.reshape(NCORES, BS, T, K), (0, 2, 1, 3))
        return wp.astype(xdt_np).reshape(NCORES * M, K)

    # host CRF + score from emissions
    def crf(em, mask, label, b_lin, start_trans, end_trans, trans):
        em = em + b_lin
        m = mask
        mf = m.astype(jnp.float32)
        em_sc = jnp.take_along_axis(em, label[..., None], -1)[..., 0]
        tr_sc = trans[label[:, :-1], label[:, 1:]]
        score = start_trans[label[:, 0]] + em_sc[:, 0] \
            + jnp.sum(mf[:, 1:] * (tr_sc + em_sc[:, 1:]), axis=1)
        last = jnp.sum(m.astype(jnp.int32), axis=1) - 1
        last_tag = jnp.take_along_axis(label, last[:, None], 1)[:, 0]
        score = score + end_trans[last_tag]

        def pstep(alpha, inp):
            em_t, m_t = inp
            nxt = jax.nn.logsumexp(
                alpha[:, :, None] + trans[None] + em_t[:, None, :], axis=1)
            return jnp.where(m_t[:, None], nxt, alpha), None
        alpha0 = start_trans + em[:, 0]
        alpha, _ = jax.lax.scan(
            pstep, alpha0, (em[:, 1:].transpose(1, 0, 2), m[:, 1:].T))
        logZ = jax.nn.logsumexp(alpha + end_trans, axis=-1)
        return -jnp.mean(score - logZ)

    with jax.default_device(cpu):
        prep_jit = jax.jit(prep_x)
        crf_jit = jax.jit(crf)

    _C.update(sharded=sharded, prep=prep_jit, crf=crf_jit, cpu=cpu, jax=jax)
    return _C


def _xdt_np():
    import ml_dtypes
    return ml_dtypes.float8_e4m3 if X_DTYPE == "fp8" else ml_dtypes.bfloat16


def _host_params(W_ih_f, W_ih_b, W_hh_f, W_hh_b, b_ih_f, b_ih_b,
                 b_hh_f, b_hh_b, W_lin):
    """Build wk [K,N], wr [128,384] f32, params [128,13] f32."""
    xdt = _xdt_np()
    wk = np.empty((K, N), np.float32)
    for c in range(3):          # r, z, n
        wk[:, c * 128:c * 128 + 64] = W_ih_f[c * 64:(c + 1) * 64, :].T
        wk[:, c * 128 + 64:(c + 1) * 128] = W_ih_b[c * 64:(c + 1) * 64, :].T
    wk = wk.astype(xdt)

    wr = np.zeros((128, 384), np.float32)
    for c in range(3):
        wr[0:64, c * 128:c * 128 + 64] = W_hh_f[c * 64:(c + 1) * 64, :].T
        wr[64:128, c * 128 + 64:(c + 1) * 128] = W_hh_b[c * 64:(c + 1) * 64, :].T

    params = np.zeros((128, 13), np.float32)
    for c in range(3):
        bf = b_ih_f[c * 64:(c + 1) * 64].copy()
        bb = b_ih_b[c * 64:(c + 1) * 64].copy()
        if c < 2:               # fold b_hh into r,z; n keeps b_ih only
            bf += b_hh_f[c * 64:(c + 1) * 64]
            bb += b_hh_b[c * 64:(c + 1) * 64]
        params[0:64, c] = bf
        params[64:128, c] = bb
    params[0:64, 3] = b_hh_f[128:192]
    params[64:128, 3] = b_hh_b[128:192]
    params[:, 4:4 + L] = W_lin.T
    ident = np.eye(128, dtype=xdt)
    return wk, wr, params, ident


def kernel(length, word2vec, mask, label, W_ih_f, W_hh_f, b_ih_f, b_hh_f,
           W_ih_b, W_hh_b, b_ih_b, b_hh_b, W_lin, b_lin,
           start_trans, end_trans, trans):
    import time as _time
    word2vec = np.asarray(word2vec, np.float32)
    mask = np.asarray(mask)
    label = np.asarray(label)
    args = [np.asarray(a, np.float32) for a in
            (W_ih_f, W_hh_f, b_ih_f, b_hh_f, W_ih_b, W_hh_b, b_ih_b, b_hh_b,
             W_lin, b_lin, start_trans, end_trans, trans)]
    (W_ih_f, W_hh_f, b_ih_f, b_hh_f, W_ih_b, W_hh_b, b_ih_b, b_hh_b,
     W_lin, b_lin, start_trans, end_trans, trans) = args

    tlog = _C.setdefault("t", {})
    try:
        t0 = _time.perf_counter()
        if "sharded" not in _C:
            _build()
        jax = _C["jax"]
        t1 = _time.perf_counter()
        import ml_dtypes
        wkb, wr, params, ident = _host_params(
            W_ih_f, W_ih_b, W_hh_f, W_hh_b, b_ih_f, b_ih_b, b_hh_f, b_hh_b,
            W_lin)
        mrow = np.ascontiguousarray(
            mask.reshape(NCORES, BS, T).transpose(0, 2, 1)
        ).reshape(NCORES, 1, M).astype(ml_dtypes.bfloat16)
        xb = _C["prep"](jax.device_put(word2vec, _C["cpu"]))  # [8*M, K] on cpu
        t2 = _time.perf_counter()
        em_dev = _C["sharded"](xb, wkb, wr, params,
                               mrow.reshape(NCORES * 1, M), ident)
        em_dev.block_until_ready()
        t3 = _time.perf_counter()
        em_np = np.asarray(em_dev)          # [8*128, 576]
        t4 = _time.perf_counter()
        em = em_np.reshape(NCORES, 128, CHUNKS, L).transpose(0, 2, 1, 3)
        em = em.reshape(NCORES, T, BS, L).transpose(0, 2, 1, 3)
        em = np.ascontiguousarray(em.reshape(B, T, L))
        t5 = _time.perf_counter()
        dp = lambda a: jax.device_put(np.asarray(a), _C["cpu"])
        loss = np.float32(_C["crf"](
            dp(em), dp(mask), dp(label), dp(b_lin), dp(start_trans),
            dp(end_trans), dp(trans)))
        t6 = _time.perf_counter()
        tlog.update(build=t1 - t0, prep=t2 - t1, device=t3 - t2,
                    fetch=t4 - t3, reorder=t5 - t4, crf=t6 - t5, dev_ok=True)
        return loss
    except Exception as e:
        tlog.update(dev_ok=False, dev_err=repr(e)[:800])
        return _full_numpy(
            word2vec, mask, label, W_ih_f, W_hh_f, b_ih_f, b_hh_f,
            W_ih_b, W_hh_b, b_ih_b, b_hh_b, W_lin, b_lin,
            start_trans, end_trans, trans)


# ---------- pure-numpy fallback (mirrors reference exactly) ----------

def _sigmoid(x):
    return 1.0 / (1.0 + np.exp(-x))


def _gru_dir_np(xp, m, W_hh, b_hh):
    Bn = xp.shape[1]
    h = np.zeros((Bn, H), np.float32)
    out = np.empty((T, Bn, H), np.float32)
    WhhT = W_hh.T.astype(np.float32)
    for t in range(T):
        hg = h @ WhhT + b_hh
        xg = xp[t]
        r = _sigmoid(xg[:, :H] + hg[:, :H])
        z = _sigmoid(xg[:, H:2 * H] + hg[:, H:2 * H])
        n = np.tanh(xg[:, 2 * H:] + r * hg[:, 2 * H:])
        h_new = (1.0 - z) * n + z * h
        mt = m[t]
        h = np.where(mt > 0, h_new, h)
        out[t] = h * mt
    return out


def _logsumexp_np(x, axis):
    mx = np.max(x, axis=axis, keepdims=True)
    return (mx + np.log(np.sum(np.exp(x - mx), axis=axis,
                               keepdims=True))).squeeze(axis)


def _full_numpy(word2vec, mask, label, W_ih_f, W_hh_f, b_ih_f, b_hh_f,
                W_ih_b, W_hh_b, b_ih_b, b_hh_b, W_lin, b_lin,
                start_trans, end_trans, trans):
    Wcat = np.concatenate([W_ih_f.T, W_ih_b.T], axis=1)
    proj = (word2vec.reshape(B * T, K) @ Wcat).reshape(B, T, 2 * G3)
    mf = mask.astype(np.float32)
    mt = mf.T[:, :, None]
    xp_f = proj[:, :, :G3].transpose(1, 0, 2) + b_ih_f
    xp_b = proj[:, :, G3:].transpose(1, 0, 2) + b_ih_b
    out_f = _gru_dir_np(xp_f, mt, W_hh_f, b_hh_f)
    out_b = _gru_dir_np(xp_b[::-1], mt[::-1], W_hh_b, b_hh_b)[::-1]
    feat = np.concatenate([out_f, out_b], -1).transpose(1, 0, 2)
    em = feat @ W_lin.T + b_lin

    em_sc = np.take_along_axis(em, label[..., None], -1)[..., 0]
    tr_sc = trans[label[:, :-1], label[:, 1:]]
    score = start_trans[label[:, 0]] + em_sc[:, 0] \
        + np.sum(mf[:, 1:] * (tr_sc + em_sc[:, 1:]), axis=1)
    last = mask.astype(np.int64).sum(1) - 1
    last_tag = label[np.arange(label.shape[0]), last]
    score = score + end_trans[last_tag]

    alpha = start_trans + em[:, 0]
    for t in range(1, T):
        nxt = _logsumexp_np(
            alpha[:, :, None] + trans[None] + em[:, t][:, None, :], axis=1)
        alpha = np.where(mask[:, t][:, None], nxt, alpha)
    logZ = _logsumexp_np(alpha + end_trans, axis=-1)
    return np.float32(-(score - logZ).mean())
